# revision 21
# baseline (speedup 1.0000x reference)
"""BinEmbedding kernel for Trainium2 (8 NeuronCores, data-parallel).

out[b, l, :] = emb_table[tok(x[b, l])]
  tok = 0 for NaN x, else clamp(searchsorted(bins, x, 'right') - 1, 0) + 1
      = [x >= -FLT_MAX] + sum_{j=1..255} [x >= bins[j]]  (exact fp32 is_ge;
        NaN compares false everywhere -> 0)

Per core: x slab [128, 1024] f32 (columns within each 128-col window are
host-permuted: dev col m holds orig col (m%8)*16 + m//8, so the gather's
16-wrap lands output rows in 4KB-contiguous DRAM runs).

VectorE: custom fused DVE ops count 4 thresholds in the first pass
(BIN_INIT4, MaxNeg standing in for the lowest threshold) and 3 per pass
after (BIN_ACC3): 85 passes per 256-col block instead of 256.

SWDGE dma_gather of 256-B table rows runs on all 4 gpsimd queues (queue q
= Q7 cpu pair 2q/2q+1; its tx cpu reads idxs from partitions 32q+16:32q+32):
  band k odd  -> queue (k-1)//2: tok rows 16k:16k+16 ARE that queue's tx
                band; idxs read in place from tok, zero copies.
  band k even -> queue k//2: one DMA per (block, k) shifts tok rows down 16
                partitions into idxb (the queue's tx band). rx-band contents
                are irrelevant (only trailing-negative trim is checked;
                tokens and zeros are both >= 0).
Gathers are paced by DMA completion: call i waits for call i-3's transfer
semaphore, so at most 3 SWDGE descgens run concurrently. 4 concurrent
descgens corrupt descriptors on this part (validated empirically: depth<=3
exact, depth 4 garbage). Pacing must use the ring-descriptor DMA sems --
prepare_only's engine EVSEM misfires under concurrency (flaky corruption),
while ring sems are ordered after descriptor generation by construction.

Call (b, k, s) covers the 128-col window C0 = b*256 + s*128; idx slot
i = m*16 + j holds tok[16k+j, C0+m]; gather dst slot i -> dst[i%128, i//128].
With the host column swap, dst[p, kk, h] maps to orig element
(16k + p%16)*1024 + C0 + (p//16)*16 + kk, so the store's DRAM AP is
[[1024, 8], [65536, 16], [1, 1024]] f32 (4KB contiguous runs).
"""

import sys

sys.path.insert(0, "/opt/trn_rl_repo")

import numpy as np

import concourse.bacc as bacc
import concourse.bass as bass
import concourse.mybir as mybir
from concourse.bass_utils import run_bass_kernel_spmd
from concourse.library_config import mlp
from concourse import dve_ops as _dve_ops
from concourse.dve_spec import C0, C1, C2, MaxNeg, Spec, Src0, Src1, lower
from concourse.dve_uop import DveOpSpec


def _register_dve_op(name: str, spec: Spec, rd1: bool) -> "_dve_ops.DveOp":
    """Register a custom DVE op at import time (idempotent). The uops sha is
    self-pinned from this process's lower() output."""
    for op in _dve_ops.OPS:
        if op.name == name:
            return op
    opcode = _dve_ops._CUSTOM_DVE_ROW_BASE + len(_dve_ops.OPS)
    assert opcode < 0x20
    shas = {}
    for ver in ("v3", "v4"):
        try:
            s = DveOpSpec(name=name, opcode=opcode, uops=lower(spec, ver=ver), rd1_en=rd1)
            shas[ver] = s.sha(ver)
        except Exception:
            pass
    op = _dve_ops.DveOp(name, spec, subdim=False, uops_sha=shas)
    _dve_ops.OPS.append(op)
    _dve_ops._SUB_OPCODE_FOR_NAME[name] = opcode
    _dve_ops.CUSTOM_DVE_SPECS[name] = spec
    return op


_FMAX = float(np.finfo(np.float32).max)

# acc = [x >= -FLT_MAX] + [x >= s0] + [x >= s1] + [x >= imm2]
BIN_INIT4 = _register_dve_op(
    "BIN_INIT4_ANT",
    Spec(
        body=(Src0 >= MaxNeg) + (Src0 >= C0) + (Src0 >= C1) + (Src0 >= C2),
        reference=lambda in0, in1, s0, s1, imm2: (
            (in0 >= -_FMAX).astype(np.float32)
            + (in0 >= s0)
            + (in0 >= s1)
            + (in0 >= imm2)
        ),
    ),
    rd1=False,
)

# acc = acc + [x >= s0] + [x >= s1] + [x >= imm2]
BIN_ACC3 = _register_dve_op(
    "BIN_ACC3_ANT",
    Spec(
        body=Src1 + (Src0 >= C0) + (Src0 >= C1) + (Src0 >= C2),
        reference=lambda in0, in1, s0, s1, imm2: in1
        + (in0 >= s0).astype(np.float32)
        + (in0 >= s1)
        + (in0 >= imm2),
    ),
    rd1=True,
)

B, L = 16, 65536
NUM_BINS = 256
H = 64
P = 128
NCORES = 8

COLS = 1024
BLOCK_COLS = 128          # DVE compute block (small -> gathers start early)
SUB_COLS = 64             # cols per gather call
NI = 16 * SUB_COLS        # idxs per gather call
NBUF = 32                 # dst ring depth (8 per queue)
GATHER_DEPTH = 3          # max concurrent SWDGE descgens (4 corrupts)


SUBS = BLOCK_COLS // SUB_COLS      # s values per (block, band)
CPB = 8 * SUBS                     # calls per block


def call_info(i):
    """Stream position i -> (b, k, s, q, n) with round-robin queues.

    Per block: for m8, for q in 0..3: k = 2q + m8%2, s = m8//2.
    q == i % 4; n is the 1-based per-queue ordinal of this call.
    """
    b, r = divmod(i, CPB)
    m8, q = divmod(r, 4)
    k = 2 * q + (m8 % 2)
    s = m8 // 2
    n = b * (CPB // 4) + m8 + 1
    return b, k, s, q, n


def build_nc(bins: np.ndarray, cols: int = COLS):
    assert bins.shape == (NUM_BINS,) and bins.dtype == np.float32
    nblocks = cols // BLOCK_COLS
    ncalls = nblocks * CPB

    thr = [-3.0e38] + [float(v) for v in bins[1:]]

    nc = bacc.Bacc("TRN2", target_bir_lowering=False, debug=False,
                   detect_race_conditions=False, num_swdge_queues=4)
    x_d = nc.dram_tensor("x", [P, cols], mybir.dt.float32, kind="ExternalInput")
    emb_d = nc.dram_tensor(
        "emb", [NUM_BINS + 1, H], mybir.dt.float32, kind="ExternalInput"
    )
    out_d = nc.dram_tensor(
        "out", [P, cols * H], mybir.dt.float32, kind="ExternalOutput"
    )

    from contextlib import ExitStack

    with ExitStack() as stack:
        ec = stack.enter_context
        x_sb = ec(nc.sbuf_tensor("x_sb", [P, cols], mybir.dt.float32))
        acc = ec(nc.sbuf_tensor("acc", [P, cols], mybir.dt.float32))
        tok = ec(nc.sbuf_tensor("tok", [P, cols], mybir.dt.int16))
        idxb = ec(nc.sbuf_tensor(
            "idxb", [P, (cols // BLOCK_COLS) * 4 * BLOCK_COLS], mybir.dt.int16))
        dst = ec(nc.sbuf_tensor("dst", [P, NBUF, NI // P, H], mybir.dt.float32))
        sem_x = ec(nc.semaphore("sem_x"))
        sem_v = ec(nc.semaphore("sem_v"))
        sem_rep = ec(nc.semaphore("sem_rep"))
        sem_z = ec(nc.semaphore("sem_z"))
        sem_gd = [ec(nc.semaphore(f"sg{t}")) for t in range(4)]
        sem_st = [ec(nc.semaphore(f"ss{t}")) for t in range(4)]
        sem_pp = [ec(nc.semaphore(f"sp{t}")) for t in range(4)]
        ni_reg = ec(nc.gpsimd.register("ni_reg"))
        block = ec(nc.Block())

        @block.vector
        def _(vector):
            # rx bands of idxb must be >= 0 for the gather's trailing-trim
            # check; zeros suffice.
            vector.memset(idxb[:, :], 0).then_inc(sem_z, 1)
            vector.wait_ge(sem_x, 16)
            for b in range(nblocks):
                lo, hi = b * BLOCK_COLS, (b + 1) * BLOCK_COLS
                xs = x_sb[:, lo:hi]
                ac = acc[:, lo:hi]
                vector._custom_dve(
                    BIN_INIT4, out=ac, in0=xs,
                    s0=thr[1], s1=thr[2], imm2=thr[3],
                )
                for j in range(4, NUM_BINS, 3):
                    vector._custom_dve(
                        BIN_ACC3, out=ac, in0=xs, in1=ac,
                        s0=thr[j], s1=thr[j + 1], imm2=thr[j + 2],
                    )
                vector.tensor_copy(tok[:, lo:hi], ac).then_inc(sem_v, 1)

        @block.scalar
        def _(scalar):
            # even bands k: shift tok rows 16k:16k+16 down 16 partitions into
            # idxb so they land in queue (k//2)'s tx band. One DMA per (b, k).
            scalar.wait_ge(sem_z, 1)
            for b in range(nblocks):
                scalar.wait_ge(sem_v, b + 1)
                for kh in range(4):       # k = 2*kh
                    k = 2 * kh
                    src_ap = tok[16 * k : 16 * k + 16,
                                 b * BLOCK_COLS : (b + 1) * BLOCK_COLS]
                    base = (b * 4 + kh) * BLOCK_COLS
                    dst_ap = idxb[16 * k + 16 : 16 * k + 32,
                                  base : base + BLOCK_COLS]
                    scalar.dma_start(dst_ap, src_ap).then_inc(sem_rep, 16)

        @block.gpsimd
        def _(gpsimd):
            gpsimd.load_library(mlp)
            gpsimd.reg_mov(ni_reg, NI)

            def idx_ap_for(i):
                b, k, s, q, n = call_info(i)
                c0 = b * BLOCK_COLS + s * SUB_COLS
                if k % 2 == 1:
                    return tok[:, c0 : c0 + SUB_COLS]
                base = (b * 4 + k // 2) * BLOCK_COLS + s * SUB_COLS
                return idxb[:, base : base + SUB_COLS]

            for i in range(ncalls):
                b, k, s, q, n = call_info(i)
                if i % CPB == 0:
                    # block b fully binned + replicated (4 DMAs x 16 incs)
                    gpsimd.wait_ge(sem_rep, 64 * (b + 1))
                if i >= NBUF:
                    gpsimd.wait_ge(sem_st[q], 16 * (n - NBUF // 4))
                if i >= GATHER_DEPTH:
                    # pace on DMA completion of the call GATHER_DEPTH back:
                    # ring-descriptor sems are inherently ordered after desc
                    # generation, unlike prep EVSEMs which misfire under
                    # concurrency (validated: prep-gating is flaky, completion
                    # gating is stable).
                    _, _, _, qd, nd = call_info(i - GATHER_DEPTH)
                    gpsimd.wait_ge(sem_gd[qd], 16 * nd)
                gpsimd.dma_gather(
                    dst[:, i % NBUF, :, :],
                    emb_d[:, :],
                    idx_ap_for(i),
                    NI,
                    ni_reg,
                    H,
                    queue_num=q,
                    single_packet=False,
                ).then_inc(sem_gd[q], 16)

        @block.sync
        def _(sync):
            sync.dma_start(x_sb[:, :], x_d[:, :]).then_inc(sem_x, 16)
            for i in range(ncalls):
                b, k, s, q, n = call_info(i)
                c0 = b * BLOCK_COLS + s * SUB_COLS
                sync.wait_ge(sem_gd[q], 16 * n)
                # dst[p, kk, h] -> orig element (16k + p%16)*1024 + c0
                #                  + (p//16)*16 + kk   (host col swap)
                out_ap = bass.AP(
                    out_d,
                    (16 * k * cols + c0) * H,
                    [
                        [(SUB_COLS // 8) * H, 8],   # u' = p//16
                        [cols * H, 16],             # j = p%16 (out rows)
                        [1, (SUB_COLS // 8) * H],   # kk*H + h contiguous
                    ],
                )
                src_ap = dst[:, i % NBUF, :, :].rearrange("p a h -> p (a h)")
                sync.dma_start(out_ap, src_ap).then_inc(sem_st[q], 16)

    nc.compile()
    return nc


_CACHE: dict = {}


def _get_nc(bins: np.ndarray):
    key = bins.tobytes()
    if key not in _CACHE:
        _CACHE[key] = build_nc(bins)
    return _CACHE[key]


def _swap_cols(slab: np.ndarray) -> np.ndarray:
    """dev col m of each window holds orig col (m%8)*(SUB_COLS//8) + m//8."""
    p, c = slab.shape
    return (
        slab.reshape(p, c // SUB_COLS, 8, SUB_COLS // 8)
        .swapaxes(2, 3)
        .reshape(p, c)
        .copy()
    )


def kernel(x: np.ndarray, bins: np.ndarray, emb_table: np.ndarray) -> np.ndarray:
    x = np.asarray(x, dtype=np.float32)
    bins = np.asarray(bins, dtype=np.float32)
    emb_table = np.asarray(emb_table, dtype=np.float32)
    assert x.shape == (B, L) and emb_table.shape == (NUM_BINS + 1, H)

    nc = _get_nc(bins)
    rows_per_core = B // NCORES
    in_maps = [
        {
            "x": _swap_cols(x[i * rows_per_core : (i + 1) * rows_per_core].reshape(P, -1)),
            "emb": emb_table,
        }
        for i in range(NCORES)
    ]
    res = run_bass_kernel_spmd(nc, in_maps, core_ids=list(range(NCORES)))
    outs = [
        res.results[i]["out"].reshape(rows_per_core, L, H) for i in range(NCORES)
    ]
    return np.concatenate(outs, axis=0)


if __name__ == "__main__":
    import concourse.bass_interp as bass_interp

    # CoreSim's gather reads idx partitions [0:16); real HW's tx cpu for
    # queue q reads [32q+16, 32q+32) (ucode: cpu_id == 2q+1 pops its own
    # 16-partition band). Patch the sim to model HW so the in-place tx-band
    # idx layout validates.
    _orig_exec = bass_interp.InstructionExecutor._exec_InstDMAGatherAnt

    def _exec_hw_bands(self, ins, captured, *, reg_snapshot):
        idxs_ap, num_idxs_reg = captured
        q = ins.queue_num
        idxs_hw = np.array(idxs_ap)
        idxs_hw[:16, :] = idxs_ap[32 * q + 16 : 32 * q + 32, :]
        return _orig_exec(
            self, ins, (idxs_hw, num_idxs_reg), reg_snapshot=reg_snapshot
        )

    bass_interp.InstructionExecutor._exec_InstDMAGatherAnt = _exec_hw_bands

    rng = np.random.default_rng(0)
    n = P * COLS
    bins = np.sort(rng.standard_normal(NUM_BINS).astype(np.float32) * 1.5)
    emb = rng.standard_normal((NUM_BINS + 1, H)).astype(np.float32)
    xs = rng.standard_normal(n).astype(np.float32)
    xs[rng.random(n) < 0.1] = np.nan

    nc = build_nc(bins)
    sim = bass_interp.CoreSim(nc, require_nnan=False, require_finite=False)
    sim.tensor("x")[:] = _swap_cols(xs.reshape(P, COLS))
    sim.tensor("emb")[:] = emb
    sim.simulate()
    got = np.asarray(sim.tensor("out")).reshape(n, H)

    nans = np.isnan(xs)
    xc = np.where(nans, 0.0, xs)
    idx = np.maximum(np.searchsorted(bins, xc, side="right") - 1, 0)
    tok_ref = np.where(nans, 0, idx + 1)
    want = emb[tok_ref]
    err = np.abs(got - want).max()
    print("sim absmax err:", err)
    print("sim time estimate:", sim.time, "ns")
    assert err == 0.0, err
    print("SIM OK")


# revision 22
# speedup vs baseline: 1.0485x; 1.0485x over previous
"""BinEmbedding kernel for Trainium2 (8 NeuronCores, data-parallel).

out[b, l, :] = emb_table[tok(x[b, l])]
  tok = 0 for NaN x, else clamp(searchsorted(bins, x, 'right') - 1, 0) + 1
      = [x >= -FLT_MAX] + sum_{j=1..255} [x >= bins[j]]  (exact fp32 is_ge;
        NaN compares false everywhere -> 0)

Per core: x slab [128, 1024] f32 (columns within each 128-col window are
host-permuted: dev col m holds orig col (m%8)*16 + m//8, so the gather's
16-wrap lands output rows in 4KB-contiguous DRAM runs).

VectorE: custom fused DVE ops count 4 thresholds in the first pass
(BIN_INIT4, MaxNeg standing in for the lowest threshold) and 3 per pass
after (BIN_ACC3): 85 passes per 256-col block instead of 256.

SWDGE dma_gather of 256-B table rows runs on all 4 gpsimd queues (queue q
= Q7 cpu pair 2q/2q+1; its tx cpu reads idxs from partitions 32q+16:32q+32):
  band k odd  -> queue (k-1)//2: tok rows 16k:16k+16 ARE that queue's tx
                band; idxs read in place from tok, zero copies.
  band k even -> queue k//2: one DMA per (block, k) shifts tok rows down 16
                partitions into idxb (the queue's tx band). rx-band contents
                are irrelevant (only trailing-negative trim is checked;
                tokens and zeros are both >= 0).
Gathers are paced by DMA completion: call i waits for call i-3's transfer
semaphore, so at most 3 SWDGE descgens run concurrently. 4 concurrent
descgens corrupt descriptors on this part (validated empirically: depth<=3
exact, depth 4 garbage). Pacing must use the ring-descriptor DMA sems --
prepare_only's engine EVSEM misfires under concurrency (flaky corruption),
while ring sems are ordered after descriptor generation by construction.

Call (b, k, s) covers the 128-col window C0 = b*256 + s*128; idx slot
i = m*16 + j holds tok[16k+j, C0+m]; gather dst slot i -> dst[i%128, i//128].
With the host column swap, dst[p, kk, h] maps to orig element
(16k + p%16)*1024 + C0 + (p//16)*16 + kk, so the store's DRAM AP is
[[1024, 8], [65536, 16], [1, 1024]] f32 (4KB contiguous runs).
"""

import sys

sys.path.insert(0, "/opt/trn_rl_repo")

import numpy as np

import concourse.bacc as bacc
import concourse.bass as bass
import concourse.mybir as mybir
from concourse.bass_utils import run_bass_kernel_spmd
from concourse.library_config import mlp
from concourse import dve_ops as _dve_ops
from concourse.dve_spec import C0, C1, C2, MaxNeg, Spec, Src0, Src1, lower
from concourse.dve_uop import DveOpSpec


def _register_dve_op(name: str, spec: Spec, rd1: bool) -> "_dve_ops.DveOp":
    """Register a custom DVE op at import time (idempotent). The uops sha is
    self-pinned from this process's lower() output."""
    for op in _dve_ops.OPS:
        if op.name == name:
            return op
    opcode = _dve_ops._CUSTOM_DVE_ROW_BASE + len(_dve_ops.OPS)
    assert opcode < 0x20
    shas = {}
    for ver in ("v3", "v4"):
        try:
            s = DveOpSpec(name=name, opcode=opcode, uops=lower(spec, ver=ver), rd1_en=rd1)
            shas[ver] = s.sha(ver)
        except Exception:
            pass
    op = _dve_ops.DveOp(name, spec, subdim=False, uops_sha=shas)
    _dve_ops.OPS.append(op)
    _dve_ops._SUB_OPCODE_FOR_NAME[name] = opcode
    _dve_ops.CUSTOM_DVE_SPECS[name] = spec
    return op


_FMAX = float(np.finfo(np.float32).max)

# acc = [x >= -FLT_MAX] + [x >= s0] + [x >= s1] + [x >= imm2]
BIN_INIT4 = _register_dve_op(
    "BIN_INIT4_ANT",
    Spec(
        body=(Src0 >= MaxNeg) + (Src0 >= C0) + (Src0 >= C1) + (Src0 >= C2),
        reference=lambda in0, in1, s0, s1, imm2: (
            (in0 >= -_FMAX).astype(np.float32)
            + (in0 >= s0)
            + (in0 >= s1)
            + (in0 >= imm2)
        ),
    ),
    rd1=False,
)

# acc = acc + [x >= s0] + [x >= s1] + [x >= imm2]
BIN_ACC3 = _register_dve_op(
    "BIN_ACC3_ANT",
    Spec(
        body=Src1 + (Src0 >= C0) + (Src0 >= C1) + (Src0 >= C2),
        reference=lambda in0, in1, s0, s1, imm2: in1
        + (in0 >= s0).astype(np.float32)
        + (in0 >= s1)
        + (in0 >= imm2),
    ),
    rd1=True,
)

B, L = 16, 65536
NUM_BINS = 256
H = 64
P = 128
NCORES = 8

COLS = 1024
BLOCK_COLS = 256          # DVE compute block
SUB_COLS = 64             # cols per gather call
NI = 16 * SUB_COLS        # idxs per gather call
NBUF = 32                 # dst ring depth (8 per queue)
GATHER_DEPTH = 3          # max concurrent SWDGE descgens (4 corrupts)


SUBS = BLOCK_COLS // SUB_COLS      # s values per (block, band)
CPB = 8 * SUBS                     # calls per block


def call_info(i):
    """Stream position i -> (b, k, s, q, n) with round-robin queues.

    Per block: for m8, for q in 0..3: k = 2q + m8%2, s = m8//2.
    q == i % 4; n is the 1-based per-queue ordinal of this call.
    """
    b, r = divmod(i, CPB)
    m8, q = divmod(r, 4)
    k = 2 * q + (m8 % 2)
    s = m8 // 2
    n = b * (CPB // 4) + m8 + 1
    return b, k, s, q, n


def build_nc(bins: np.ndarray, cols: int = COLS):
    assert bins.shape == (NUM_BINS,) and bins.dtype == np.float32
    nblocks = cols // BLOCK_COLS
    ncalls = nblocks * CPB

    thr = [-3.0e38] + [float(v) for v in bins[1:]]

    nc = bacc.Bacc("TRN2", target_bir_lowering=False, debug=False,
                   detect_race_conditions=False, num_swdge_queues=4)
    x_d = nc.dram_tensor("x", [P, cols], mybir.dt.float32, kind="ExternalInput")
    emb_d = nc.dram_tensor(
        "emb", [NUM_BINS + 1, H], mybir.dt.float32, kind="ExternalInput"
    )
    out_d = nc.dram_tensor(
        "out", [P, cols * H], mybir.dt.float32, kind="ExternalOutput"
    )

    from contextlib import ExitStack

    with ExitStack() as stack:
        ec = stack.enter_context
        x_sb = ec(nc.sbuf_tensor("x_sb", [P, cols], mybir.dt.float32))
        acc = ec(nc.sbuf_tensor("acc", [P, cols], mybir.dt.float32))
        tok = ec(nc.sbuf_tensor("tok", [P, cols], mybir.dt.int16))
        idxb = ec(nc.sbuf_tensor(
            "idxb", [P, (cols // BLOCK_COLS) * 4 * BLOCK_COLS], mybir.dt.int16))
        dst = ec(nc.sbuf_tensor("dst", [P, NBUF, NI // P, H], mybir.dt.float32))
        sem_x = ec(nc.semaphore("sem_x"))
        sem_v = ec(nc.semaphore("sem_v"))
        sem_rep = ec(nc.semaphore("sem_rep"))
        sem_z = ec(nc.semaphore("sem_z"))
        sem_gd = [ec(nc.semaphore(f"sg{t}")) for t in range(4)]
        sem_st = [ec(nc.semaphore(f"ss{t}")) for t in range(4)]
        sem_pp = [ec(nc.semaphore(f"sp{t}")) for t in range(4)]
        ni_reg = ec(nc.gpsimd.register("ni_reg"))
        block = ec(nc.Block())

        @block.vector
        def _(vector):
            # rx bands of idxb must be >= 0 for the gather's trailing-trim
            # check; zeros suffice.
            vector.memset(idxb[:, :], 0).then_inc(sem_z, 1)
            vector.wait_ge(sem_x, 16)
            for b in range(nblocks):
                lo, hi = b * BLOCK_COLS, (b + 1) * BLOCK_COLS
                xs = x_sb[:, lo:hi]
                ac = acc[:, lo:hi]
                vector._custom_dve(
                    BIN_INIT4, out=ac, in0=xs,
                    s0=thr[1], s1=thr[2], imm2=thr[3],
                )
                for j in range(4, NUM_BINS, 3):
                    vector._custom_dve(
                        BIN_ACC3, out=ac, in0=xs, in1=ac,
                        s0=thr[j], s1=thr[j + 1], imm2=thr[j + 2],
                    )
                vector.tensor_copy(tok[:, lo:hi], ac).then_inc(sem_v, 1)

        @block.scalar
        def _(scalar):
            # even bands k: shift tok rows 16k:16k+16 down 16 partitions into
            # idxb so they land in queue (k//2)'s tx band. One DMA per (b, k).
            scalar.wait_ge(sem_z, 1)
            for b in range(nblocks):
                scalar.wait_ge(sem_v, b + 1)
                for kh in range(4):       # k = 2*kh
                    k = 2 * kh
                    src_ap = tok[16 * k : 16 * k + 16,
                                 b * BLOCK_COLS : (b + 1) * BLOCK_COLS]
                    base = (b * 4 + kh) * BLOCK_COLS
                    dst_ap = idxb[16 * k + 16 : 16 * k + 32,
                                  base : base + BLOCK_COLS]
                    scalar.dma_start(dst_ap, src_ap).then_inc(sem_rep, 16)

        @block.gpsimd
        def _(gpsimd):
            gpsimd.load_library(mlp)
            gpsimd.reg_mov(ni_reg, NI)

            def idx_ap_for(i):
                b, k, s, q, n = call_info(i)
                c0 = b * BLOCK_COLS + s * SUB_COLS
                if k % 2 == 1:
                    return tok[:, c0 : c0 + SUB_COLS]
                base = (b * 4 + k // 2) * BLOCK_COLS + s * SUB_COLS
                return idxb[:, base : base + SUB_COLS]

            for i in range(ncalls):
                b, k, s, q, n = call_info(i)
                if i % CPB == 0:
                    # block b fully binned + replicated (4 DMAs x 16 incs)
                    gpsimd.wait_ge(sem_rep, 64 * (b + 1))
                if i >= NBUF:
                    gpsimd.wait_ge(sem_st[q], 16 * (n - NBUF // 4))
                if i >= GATHER_DEPTH:
                    # pace on DMA completion of the call GATHER_DEPTH back:
                    # ring-descriptor sems are inherently ordered after desc
                    # generation, unlike prep EVSEMs which misfire under
                    # concurrency (validated: prep-gating is flaky, completion
                    # gating is stable).
                    _, _, _, qd, nd = call_info(i - GATHER_DEPTH)
                    gpsimd.wait_ge(sem_gd[qd], 16 * nd)
                gpsimd.dma_gather(
                    dst[:, i % NBUF, :, :],
                    emb_d[:, :],
                    idx_ap_for(i),
                    NI,
                    ni_reg,
                    H,
                    queue_num=q,
                    single_packet=False,
                ).then_inc(sem_gd[q], 16)

        @block.sync
        def _(sync):
            sync.dma_start(x_sb[:, :], x_d[:, :]).then_inc(sem_x, 16)
            for i in range(ncalls):
                b, k, s, q, n = call_info(i)
                c0 = b * BLOCK_COLS + s * SUB_COLS
                sync.wait_ge(sem_gd[q], 16 * n)
                # dst[p, kk, h] -> orig element (16k + p%16)*1024 + c0
                #                  + (p//16)*16 + kk   (host col swap)
                out_ap = bass.AP(
                    out_d,
                    (16 * k * cols + c0) * H,
                    [
                        [(SUB_COLS // 8) * H, 8],   # u' = p//16
                        [cols * H, 16],             # j = p%16 (out rows)
                        [1, (SUB_COLS // 8) * H],   # kk*H + h contiguous
                    ],
                )
                src_ap = dst[:, i % NBUF, :, :].rearrange("p a h -> p (a h)")
                sync.dma_start(out_ap, src_ap).then_inc(sem_st[q], 16)

    nc.compile()
    return nc


_CACHE: dict = {}


def _get_nc(bins: np.ndarray):
    key = bins.tobytes()
    if key not in _CACHE:
        _CACHE[key] = build_nc(bins)
    return _CACHE[key]


def _swap_cols(slab: np.ndarray) -> np.ndarray:
    """dev col m of each window holds orig col (m%8)*(SUB_COLS//8) + m//8."""
    p, c = slab.shape
    return (
        slab.reshape(p, c // SUB_COLS, 8, SUB_COLS // 8)
        .swapaxes(2, 3)
        .reshape(p, c)
        .copy()
    )


def kernel(x: np.ndarray, bins: np.ndarray, emb_table: np.ndarray) -> np.ndarray:
    x = np.asarray(x, dtype=np.float32)
    bins = np.asarray(bins, dtype=np.float32)
    emb_table = np.asarray(emb_table, dtype=np.float32)
    assert x.shape == (B, L) and emb_table.shape == (NUM_BINS + 1, H)

    nc = _get_nc(bins)
    rows_per_core = B // NCORES
    in_maps = [
        {
            "x": _swap_cols(x[i * rows_per_core : (i + 1) * rows_per_core].reshape(P, -1)),
            "emb": emb_table,
        }
        for i in range(NCORES)
    ]
    res = run_bass_kernel_spmd(nc, in_maps, core_ids=list(range(NCORES)))
    outs = [
        res.results[i]["out"].reshape(rows_per_core, L, H) for i in range(NCORES)
    ]
    return np.concatenate(outs, axis=0)


if __name__ == "__main__":
    import concourse.bass_interp as bass_interp

    # CoreSim's gather reads idx partitions [0:16); real HW's tx cpu for
    # queue q reads [32q+16, 32q+32) (ucode: cpu_id == 2q+1 pops its own
    # 16-partition band). Patch the sim to model HW so the in-place tx-band
    # idx layout validates.
    _orig_exec = bass_interp.InstructionExecutor._exec_InstDMAGatherAnt

    def _exec_hw_bands(self, ins, captured, *, reg_snapshot):
        idxs_ap, num_idxs_reg = captured
        q = ins.queue_num
        idxs_hw = np.array(idxs_ap)
        idxs_hw[:16, :] = idxs_ap[32 * q + 16 : 32 * q + 32, :]
        return _orig_exec(
            self, ins, (idxs_hw, num_idxs_reg), reg_snapshot=reg_snapshot
        )

    bass_interp.InstructionExecutor._exec_InstDMAGatherAnt = _exec_hw_bands

    rng = np.random.default_rng(0)
    n = P * COLS
    bins = np.sort(rng.standard_normal(NUM_BINS).astype(np.float32) * 1.5)
    emb = rng.standard_normal((NUM_BINS + 1, H)).astype(np.float32)
    xs = rng.standard_normal(n).astype(np.float32)
    xs[rng.random(n) < 0.1] = np.nan

    nc = build_nc(bins)
    sim = bass_interp.CoreSim(nc, require_nnan=False, require_finite=False)
    sim.tensor("x")[:] = _swap_cols(xs.reshape(P, COLS))
    sim.tensor("emb")[:] = emb
    sim.simulate()
    got = np.asarray(sim.tensor("out")).reshape(n, H)

    nans = np.isnan(xs)
    xc = np.where(nans, 0.0, xs)
    idx = np.maximum(np.searchsorted(bins, xc, side="right") - 1, 0)
    tok_ref = np.where(nans, 0, idx + 1)
    want = emb[tok_ref]
    err = np.abs(got - want).max()
    print("sim absmax err:", err)
    print("sim time estimate:", sim.time, "ns")
    assert err == 0.0, err
    print("SIM OK")


# revision 23
# speedup vs baseline: 1.2242x; 1.1675x over previous
"""BinEmbedding kernel for Trainium2 (8 NeuronCores, data-parallel).

out[b, l, :] = emb_table[tok(x[b, l])]
  tok = 0 for NaN x, else clamp(searchsorted(bins, x, 'right') - 1, 0) + 1
      = [x >= -FLT_MAX] + sum_{j=1..255} [x >= bins[j]]  (exact fp32 is_ge;
        NaN compares false everywhere -> 0)

Per core: x slab [128, 1024] f32 (columns within each 128-col window are
host-permuted: dev col m holds orig col (m%8)*16 + m//8, so the gather's
16-wrap lands output rows in 4KB-contiguous DRAM runs).

VectorE: custom fused DVE ops count 4 thresholds in the first pass
(BIN_INIT4, MaxNeg standing in for the lowest threshold) and 3 per pass
after (BIN_ACC3): 85 passes per 256-col block instead of 256.

SWDGE dma_gather of 256-B table rows runs on all 4 gpsimd queues (queue q
= Q7 cpu pair 2q/2q+1; its tx cpu reads idxs from partitions 32q+16:32q+32):
  band k odd  -> queue (k-1)//2: tok rows 16k:16k+16 ARE that queue's tx
                band; idxs read in place from tok, zero copies.
  band k even -> queue k//2: one DMA per (block, k) shifts tok rows down 16
                partitions into idxb (the queue's tx band). rx-band contents
                are irrelevant (only trailing-negative trim is checked;
                tokens and zeros are both >= 0).
Gathers are paced by DMA completion: call i waits for call i-3's transfer
semaphore, so at most 3 SWDGE descgens run concurrently. 4 concurrent
descgens corrupt descriptors on this part (validated empirically: depth<=3
exact, depth 4 garbage). Pacing must use the ring-descriptor DMA sems --
prepare_only's engine EVSEM misfires under concurrency (flaky corruption),
while ring sems are ordered after descriptor generation by construction.

Call (b, k, s) covers the 128-col window C0 = b*256 + s*128; idx slot
i = m*16 + j holds tok[16k+j, C0+m]; gather dst slot i -> dst[i%128, i//128].
With the host column swap, dst[p, kk, h] maps to orig element
(16k + p%16)*1024 + C0 + (p//16)*16 + kk, so the store's DRAM AP is
[[1024, 8], [65536, 16], [1, 1024]] f32 (4KB contiguous runs).
"""

import sys

sys.path.insert(0, "/opt/trn_rl_repo")

import numpy as np

import concourse.bacc as bacc
import concourse.bass as bass
import concourse.mybir as mybir
from concourse.bass_utils import run_bass_kernel_spmd
from concourse.library_config import mlp
from concourse import dve_ops as _dve_ops
from concourse.dve_spec import C0, C1, C2, MaxNeg, Spec, Src0, Src1, lower
from concourse.dve_uop import DveOpSpec


def _register_dve_op(name: str, spec: Spec, rd1: bool) -> "_dve_ops.DveOp":
    """Register a custom DVE op at import time (idempotent). The uops sha is
    self-pinned from this process's lower() output."""
    for op in _dve_ops.OPS:
        if op.name == name:
            return op
    opcode = _dve_ops._CUSTOM_DVE_ROW_BASE + len(_dve_ops.OPS)
    assert opcode < 0x20
    shas = {}
    for ver in ("v3", "v4"):
        try:
            s = DveOpSpec(name=name, opcode=opcode, uops=lower(spec, ver=ver), rd1_en=rd1)
            shas[ver] = s.sha(ver)
        except Exception:
            pass
    op = _dve_ops.DveOp(name, spec, subdim=False, uops_sha=shas)
    _dve_ops.OPS.append(op)
    _dve_ops._SUB_OPCODE_FOR_NAME[name] = opcode
    _dve_ops.CUSTOM_DVE_SPECS[name] = spec
    return op


_FMAX = float(np.finfo(np.float32).max)

# acc = [x >= -FLT_MAX] + [x >= s0] + [x >= s1] + [x >= imm2]
BIN_INIT4 = _register_dve_op(
    "BIN_INIT4_ANT",
    Spec(
        body=(Src0 >= MaxNeg) + (Src0 >= C0) + (Src0 >= C1) + (Src0 >= C2),
        reference=lambda in0, in1, s0, s1, imm2: (
            (in0 >= -_FMAX).astype(np.float32)
            + (in0 >= s0)
            + (in0 >= s1)
            + (in0 >= imm2)
        ),
    ),
    rd1=False,
)

# acc = acc + [x >= s0] + [x >= s1] + [x >= imm2]
BIN_ACC3 = _register_dve_op(
    "BIN_ACC3_ANT",
    Spec(
        body=Src1 + (Src0 >= C0) + (Src0 >= C1) + (Src0 >= C2),
        reference=lambda in0, in1, s0, s1, imm2: in1
        + (in0 >= s0).astype(np.float32)
        + (in0 >= s1)
        + (in0 >= imm2),
    ),
    rd1=True,
)

B, L = 16, 65536
NUM_BINS = 256
H = 64
P = 128
NCORES = 8

COLS = 1024
BLOCK_COLS = 256          # DVE compute block
SUB_COLS = 64             # cols per gather call
NI = 16 * SUB_COLS        # idxs per gather call
NBUF = 32                 # dst ring depth (8 per queue)
GATHER_DEPTH = 3          # max concurrent SWDGE descgens (4 corrupts)


SUBS = BLOCK_COLS // SUB_COLS      # s values per (block, band)
CPB = 8 * SUBS                     # calls per block


def call_info(i):
    """Stream position i -> (b, k, s, q, n) with round-robin queues.

    Per block: for m8, for q in 0..3: k = 2q + m8%2, s = m8//2.
    q == i % 4; n is the 1-based per-queue ordinal of this call.
    """
    b, r = divmod(i, CPB)
    m8, q = divmod(r, 4)
    k = 2 * q + (m8 % 2)
    s = m8 // 2
    n = b * (CPB // 4) + m8 + 1
    return b, k, s, q, n


def build_nc(bins: np.ndarray, cols: int = COLS):
    assert bins.shape == (NUM_BINS,) and bins.dtype == np.float32
    nblocks = cols // BLOCK_COLS
    ncalls = nblocks * CPB

    thr = [-3.0e38] + [float(v) for v in bins[1:]]

    nc = bacc.Bacc("TRN2", target_bir_lowering=False, debug=False,
                   detect_race_conditions=False, num_swdge_queues=4)
    x_d = nc.dram_tensor("x", [P, cols], mybir.dt.float32, kind="ExternalInput")
    emb_d = nc.dram_tensor(
        "emb", [NUM_BINS + 1, H], mybir.dt.float32, kind="ExternalInput"
    )
    out_d = nc.dram_tensor(
        "out", [P, cols * H], mybir.dt.float32, kind="ExternalOutput"
    )

    from contextlib import ExitStack

    with ExitStack() as stack:
        ec = stack.enter_context
        x_sb = ec(nc.sbuf_tensor("x_sb", [P, cols], mybir.dt.float32))
        acc = ec(nc.sbuf_tensor("acc", [P, cols], mybir.dt.float32))
        tok = ec(nc.sbuf_tensor("tok", [P, cols], mybir.dt.int16))
        idxb = ec(nc.sbuf_tensor(
            "idxb", [P, (cols // BLOCK_COLS) * 4 * BLOCK_COLS], mybir.dt.int16))
        dst = ec(nc.sbuf_tensor("dst", [P, NBUF, NI // P, H], mybir.dt.float32))
        sem_x = ec(nc.semaphore("sem_x"))
        sem_v = ec(nc.semaphore("sem_v"))
        sem_rep = ec(nc.semaphore("sem_rep"))
        sem_z = ec(nc.semaphore("sem_z"))
        sem_gd = [ec(nc.semaphore(f"sg{t}")) for t in range(4)]
        sem_st = [ec(nc.semaphore(f"ss{t}")) for t in range(4)]
        sem_pp = [ec(nc.semaphore(f"sp{t}")) for t in range(4)]
        ni_reg = ec(nc.gpsimd.register("ni_reg"))
        block = ec(nc.Block())

        @block.vector
        def _(vector):
            # rx bands of idxb must be >= 0 for the gather's trailing-trim
            # check; zeros suffice.
            vector.memset(idxb[:, :], 0).then_inc(sem_z, 1)
            vector.wait_ge(sem_x, 16)
            for b in range(nblocks):
                lo, hi = b * BLOCK_COLS, (b + 1) * BLOCK_COLS
                xs = x_sb[:, lo:hi]
                ac = acc[:, lo:hi]
                vector._custom_dve(
                    BIN_INIT4, out=ac, in0=xs,
                    s0=thr[1], s1=thr[2], imm2=thr[3],
                )
                for j in range(4, NUM_BINS, 3):
                    vector._custom_dve(
                        BIN_ACC3, out=ac, in0=xs, in1=ac,
                        s0=thr[j], s1=thr[j + 1], imm2=thr[j + 2],
                    )
                vector.tensor_copy(tok[:, lo:hi], ac).then_inc(sem_v, 1)

        @block.scalar
        def _(scalar):
            # even bands k: shift tok rows 16k:16k+16 down 16 partitions into
            # idxb so they land in queue (k//2)'s tx band. One DMA per (b, k).
            scalar.wait_ge(sem_z, 1)
            for b in range(nblocks):
                scalar.wait_ge(sem_v, b + 1)
                for kh in range(4):       # k = 2*kh
                    k = 2 * kh
                    src_ap = tok[16 * k : 16 * k + 16,
                                 b * BLOCK_COLS : (b + 1) * BLOCK_COLS]
                    base = (b * 4 + kh) * BLOCK_COLS
                    dst_ap = idxb[16 * k + 16 : 16 * k + 32,
                                  base : base + BLOCK_COLS]
                    scalar.dma_start(dst_ap, src_ap).then_inc(sem_rep, 16)

        @block.gpsimd
        def _(gpsimd):
            gpsimd.load_library(mlp)
            gpsimd.reg_mov(ni_reg, NI)

            def idx_ap_for(i):
                b, k, s, q, n = call_info(i)
                c0 = b * BLOCK_COLS + s * SUB_COLS
                if k % 2 == 1:
                    return tok[:, c0 : c0 + SUB_COLS]
                base = (b * 4 + k // 2) * BLOCK_COLS + s * SUB_COLS
                return idxb[:, base : base + SUB_COLS]

            for i in range(ncalls):
                b, k, s, q, n = call_info(i)
                if i % CPB == 0:
                    # block b fully binned + replicated (4 DMAs x 16 incs)
                    gpsimd.wait_ge(sem_rep, 64 * (b + 1))
                if i >= NBUF:
                    gpsimd.wait_ge(sem_st[q], 16 * (n - NBUF // 4))
                if i >= GATHER_DEPTH:
                    # pace on DMA-ring sems of the call GATHER_DEPTH back:
                    # ring-descriptor sems are inherently ordered after desc
                    # generation, unlike prep EVSEMs which misfire under
                    # concurrency (validated: prep-gating is flaky, completion
                    # gating is stable). Waiting for just ONE of the call's 16
                    # ring incs suffices: calls <= nd-1 contribute at most
                    # 16*(nd-1) incs on this queue, so +1 more proves call
                    # nd's doorbell fired, i.e. its descgen is complete --
                    # without waiting out the full 16-ring drain + sem
                    # propagation (~3 us less slack in the pacing loop).
                    _, _, _, qd, nd = call_info(i - GATHER_DEPTH)
                    gpsimd.wait_ge(sem_gd[qd], 16 * (nd - 1) + 1)
                gpsimd.dma_gather(
                    dst[:, i % NBUF, :, :],
                    emb_d[:, :],
                    idx_ap_for(i),
                    NI,
                    ni_reg,
                    H,
                    queue_num=q,
                    single_packet=False,
                ).then_inc(sem_gd[q], 16)

        @block.sync
        def _(sync):
            sync.dma_start(x_sb[:, :], x_d[:, :]).then_inc(sem_x, 16)
            for i in range(ncalls):
                b, k, s, q, n = call_info(i)
                c0 = b * BLOCK_COLS + s * SUB_COLS
                sync.wait_ge(sem_gd[q], 16 * n)
                # dst[p, kk, h] -> orig element (16k + p%16)*1024 + c0
                #                  + (p//16)*16 + kk   (host col swap)
                out_ap = bass.AP(
                    out_d,
                    (16 * k * cols + c0) * H,
                    [
                        [(SUB_COLS // 8) * H, 8],   # u' = p//16
                        [cols * H, 16],             # j = p%16 (out rows)
                        [1, (SUB_COLS // 8) * H],   # kk*H + h contiguous
                    ],
                )
                src_ap = dst[:, i % NBUF, :, :].rearrange("p a h -> p (a h)")
                sync.dma_start(out_ap, src_ap).then_inc(sem_st[q], 16)

    nc.compile()
    return nc


_CACHE: dict = {}


def _get_nc(bins: np.ndarray):
    key = bins.tobytes()
    if key not in _CACHE:
        _CACHE[key] = build_nc(bins)
    return _CACHE[key]


def _swap_cols(slab: np.ndarray) -> np.ndarray:
    """dev col m of each window holds orig col (m%8)*(SUB_COLS//8) + m//8."""
    p, c = slab.shape
    return (
        slab.reshape(p, c // SUB_COLS, 8, SUB_COLS // 8)
        .swapaxes(2, 3)
        .reshape(p, c)
        .copy()
    )


def kernel(x: np.ndarray, bins: np.ndarray, emb_table: np.ndarray) -> np.ndarray:
    x = np.asarray(x, dtype=np.float32)
    bins = np.asarray(bins, dtype=np.float32)
    emb_table = np.asarray(emb_table, dtype=np.float32)
    assert x.shape == (B, L) and emb_table.shape == (NUM_BINS + 1, H)

    nc = _get_nc(bins)
    rows_per_core = B // NCORES
    in_maps = [
        {
            "x": _swap_cols(x[i * rows_per_core : (i + 1) * rows_per_core].reshape(P, -1)),
            "emb": emb_table,
        }
        for i in range(NCORES)
    ]
    res = run_bass_kernel_spmd(nc, in_maps, core_ids=list(range(NCORES)))
    outs = [
        res.results[i]["out"].reshape(rows_per_core, L, H) for i in range(NCORES)
    ]
    return np.concatenate(outs, axis=0)


if __name__ == "__main__":
    import concourse.bass_interp as bass_interp

    # CoreSim's gather reads idx partitions [0:16); real HW's tx cpu for
    # queue q reads [32q+16, 32q+32) (ucode: cpu_id == 2q+1 pops its own
    # 16-partition band). Patch the sim to model HW so the in-place tx-band
    # idx layout validates.
    _orig_exec = bass_interp.InstructionExecutor._exec_InstDMAGatherAnt

    def _exec_hw_bands(self, ins, captured, *, reg_snapshot):
        idxs_ap, num_idxs_reg = captured
        q = ins.queue_num
        idxs_hw = np.array(idxs_ap)
        idxs_hw[:16, :] = idxs_ap[32 * q + 16 : 32 * q + 32, :]
        return _orig_exec(
            self, ins, (idxs_hw, num_idxs_reg), reg_snapshot=reg_snapshot
        )

    bass_interp.InstructionExecutor._exec_InstDMAGatherAnt = _exec_hw_bands

    rng = np.random.default_rng(0)
    n = P * COLS
    bins = np.sort(rng.standard_normal(NUM_BINS).astype(np.float32) * 1.5)
    emb = rng.standard_normal((NUM_BINS + 1, H)).astype(np.float32)
    xs = rng.standard_normal(n).astype(np.float32)
    xs[rng.random(n) < 0.1] = np.nan

    nc = build_nc(bins)
    sim = bass_interp.CoreSim(nc, require_nnan=False, require_finite=False)
    sim.tensor("x")[:] = _swap_cols(xs.reshape(P, COLS))
    sim.tensor("emb")[:] = emb
    sim.simulate()
    got = np.asarray(sim.tensor("out")).reshape(n, H)

    nans = np.isnan(xs)
    xc = np.where(nans, 0.0, xs)
    idx = np.maximum(np.searchsorted(bins, xc, side="right") - 1, 0)
    tok_ref = np.where(nans, 0, idx + 1)
    want = emb[tok_ref]
    err = np.abs(got - want).max()
    print("sim absmax err:", err)
    print("sim time estimate:", sim.time, "ns")
    assert err == 0.0, err
    print("SIM OK")
